# revision 37
# baseline (speedup 1.0000x reference)
"""Causal attention (B=4, S=4096, D=64, fp32) on 8 Trainium2 NeuronCores.

Strategy
--------
Sharding: 2 cores per batch element; the two cores of a batch split the KV
blocks by parity (even / odd 128-row blocks). Each core computes, for every
query position of its batch, the *unnormalized* attention numerator and the
softmax denominator contribution of its own KV half. The host sums the two
halves and divides (exactly linear, since the softmax uses no max-subtraction:
scores/8 are bounded by ~|6| for N(0,1) inputs, so exp never overflows fp32).

Per-core device kernel (identical SPMD program; per-core behavior comes only
from input data):
  - scores^T layout: S_T[kv, q] = K @ Q^T in fp8 e4m3 (fp16 for the short-row
    q-tile 0, whose softmax rows are too short to average out fp8 score
    noise). Q^T/K^T are duplicated onto partitions 64-127 so each matmul pair
    runs concurrently in the 128x128 array via row tiling (tile_position).
    K^T is pre-scaled by log2(e) on the host so both exp paths below get a
    free affine. fp8 halves the dominant input bytes (the serial per-ring DMA
    bandwidth, ~60 GB/s, gates the pipeline start).
  - softmax exp is SPLIT across two engines (single-engine exp was the
    baseline bottleneck at ~42us busy):
      * ACT (scalar): exact exp via activation (scale folds the /8 and the
        log2e pre-scale), fp32 PSUM in -> fp16 SBUF out. Handles boundary
        (diagonal) pairs + every other interior pair. The exp table is
        preloaded during the DMA window by a dummy activation.
      * DVE (vector): Schraudolph exp for the other interior pairs:
        int16 = sc * 128 + 1024*(15+c)  (one tensor_scalar mult+add, fp32
        PSUM in), bitcast to fp16 = 2^y with linearly-interpolated mantissa
        (max ~3% relative error; averages out over the >=512-key rows that
        interior pairs serve; end-to-end rel err ~1.1e-2 vs 2e-2 budget,
        fp8 scores included).
  - the PE stream is SOFTWARE-PIPELINED with lookahead 4: the PE queue is
    in-order, so emitting mm2(i) right after exp(i) would stall the queue
    and serialize the two exp engines. Instead mm1(i+4) is emitted before
    mm2(i), so both exp engines run concurrently while the PE streams at
    ~94% occupancy (~870ns per KV pair: 3x512-col streams + exposed
    LDWEIGHTS).
  - causal masking: the two KV blocks straddling the diagonal get one
    multiplicative 0/1 fp16 mask (2x DVE mode) applied to P after the exp;
    tiles are processed deepest-first with the boundary pair last, which
    relaxes the mask-DMA deadline to mid-kernel.
  - numerator+denominator: matmul(lhsT=[V/64 | 1/64] block [128,65], rhs=P
    [128,512]) accumulated over KV blocks in PSUM; row 64 is sum(P)/64 = the
    scaled softmax denominator. The 1/64 scale keeps the fp16 output DMA
    (half the bytes of fp32) from overflowing. Padded keys are zeroed in V
    AND the ones column.
  - input DMAs are spread over six queues (sync + scalar HWDGE, 4 SWDGE
    rings -- SWDGE siblings selected by renaming the queue on the emitted
    instruction) and ordered by first use.
Host: transposes Q/K, packs per-core inputs, combines/normalizes outputs.
"""

import math
import numpy as np
from contextlib import ExitStack

import concourse.tile as tile
from concourse import bacc, mybir
from concourse.bass_utils import run_bass_kernel_spmd

B, S, D = 4, 4096, 64
NCORES = 8
BLK = 128            # kv block rows
QTW = 512            # q tile width
NQT = S // QTW       # 8 q tiles
PAR = S // BLK // 2  # 16 kv blocks per parity half
WARMUP_MMS = 3       # dummy matmuls to open the PE HAM clock gate at startup
LOOKAHEAD = 4        # software pipeline depth (mm1 slots ahead of mm2)

KSCALE = math.log2(math.e)          # host pre-scale on K^T
ACT_SCALE = 0.125 / KSCALE          # activation: exp(sc * ACT_SCALE) = exp(s/8)
SCH_C = -0.0434
SCH_A = 128.0                       # 1024*log2e/8 / KSCALE
SCH_B = 1024.0 * (15.0 + SCH_C)

TILE_ORDER = [7, 6, 5, 4, 3, 2, 0, 1]
MKW = QTW + QTW // 2  # trimmed mask width (the middle 256 cols were never read)

_prog_cache = {}


def _build_program():
    if "nc" in _prog_cache:
        return _prog_cache["nc"]
    nc = bacc.Bacc(
        "TRN2", target_bir_lowering=False, debug=False, num_devices=NCORES,
        num_swdge_queues=4,
    )
    f32, f16, i16 = mybir.dt.float32, mybir.dt.float16, mybir.dt.int16
    Exp = mybir.ActivationFunctionType.Exp

    f8 = mybir.dt.float8e4
    qt_d = nc.dram_tensor("qt", [2 * D, S], f8, kind="ExternalInput").ap()
    kt_d = nc.dram_tensor("kt", [2 * D, PAR * BLK], f8, kind="ExternalInput").ap()
    qt0_d = nc.dram_tensor("qt0", [2 * D, QTW], f16, kind="ExternalInput").ap()
    kt0_d = nc.dram_tensor("kt0", [2 * D, 2 * BLK], f16, kind="ExternalInput").ap()
    vp_d = nc.dram_tensor("vp", [BLK, PAR * 65], f16, kind="ExternalInput").ap()
    mk_d = nc.dram_tensor("mk", [BLK, MKW], f16, kind="ExternalInput").ap()
    out_d = nc.dram_tensor("out", [65, S], f16, kind="ExternalOutput").ap()

    def gdma(q, out, in_):
        # SWDGE multi-queue: dma_start pins queue 0 ("qPoolDynamic"); route
        # to a sibling ring by renaming the queue on the emitted instruction
        # (queues 1-3 are declared via num_swdge_queues=4).
        inst = nc.gpsimd.dma_start(out, in_)
        if q:
            inst.ins.queue = f"qPoolDynamic{q}"
        return inst

    with tile.TileContext(nc) as tc, ExitStack() as ctx:
        const = ctx.enter_context(tc.tile_pool(name="const", bufs=1))
        ppool = ctx.enter_context(tc.tile_pool(name="pp", bufs=7))
        opool = ctx.enter_context(tc.tile_pool(name="op", bufs=3))
        sc_ps = ctx.enter_context(tc.tile_pool(name="scps", bufs=3, space="PSUM"))
        out_ps = ctx.enter_context(tc.tile_pool(name="ops", bufs=2, space="PSUM"))

        # Input DMAs spread over six queues (sync HWDGE, scalar HWDGE, and
        # four SWDGE rings), ordered by first use: tiles run deepest-first,
        # so kt/vp prefixes and qt[t7] gate the pipeline start; the mask is
        # only needed at each tile's final (boundary) pair.
        mk_s = const.tile([BLK, MKW], f16)
        kt_s = const.tile([2 * D, PAR * BLK], f8)
        vp_s = const.tile([BLK, PAR * 65], f16)
        qt_s = const.tile([2 * D, S], f8)
        qt0_s = const.tile([2 * D, QTW], f16)
        kt0_s = const.tile([2 * D, 2 * BLK], f16)

        def qtchunk(t):
            return qt_s[:, t * QTW : (t + 1) * QTW], qt_d[:, t * QTW : (t + 1) * QTW]

        nc.scalar.dma_start(kt_s[:, 0:512], kt_d[:, 0:512])
        nc.sync.dma_start(*qtchunk(7))
        gdma(1, vp_s[:, 0 : 8 * 65], vp_d[:, 0 : 8 * 65])
        gdma(2, kt_s[:, 1280:2048], kt_d[:, 1280:2048])
        gdma(3, vp_s[:, 8 * 65 :], vp_d[:, 8 * 65 :])
        nc.scalar.dma_start(kt_s[:, 512:1280], kt_d[:, 512:1280])
        nc.sync.dma_start(*qtchunk(6))
        gdma(2, *qtchunk(5))
        gdma(1, *qtchunk(4))
        nc.scalar.dma_start(mk_s[:], mk_d[:])
        nc.sync.dma_start(*qtchunk(3))
        gdma(3, *qtchunk(2))
        gdma(2, *qtchunk(1))
        nc.sync.dma_start(*qtchunk(0))
        gdma(1, qt0_s[:], qt0_d[:])
        gdma(3, kt0_s[:], kt0_d[:])

        # PE warmup (HAM clock gate needs ~3.4us busy) + ACT exp-table
        # preload, both during the input-DMA window.
        wsrc = const.tile([BLK, QTW], f16, name="wsrc")
        nc.vector.memset(wsrc[:], 0.0)
        wact = const.tile([BLK, 8], f16, name="wact")
        nc.scalar.activation(wact[:], wsrc[:, 0:8], Exp, scale=ACT_SCALE)
        wps = sc_ps.tile([BLK, 2 * QTW], f32, tag="sc", name="wps")
        for _ in range(WARMUP_MMS):
            nc.tensor.matmul(wps[:, 0:QTW], wsrc[:, 0:BLK], wsrc[:], start=True, stop=True)

        # Flat pair work list across tiles. Each entry:
        # (T, lo, boundary, engine, first_of_tile, last_of_tile, depth)
        work = []
        for T in TILE_ORDER:
            depth = 2 * T + 2
            body = list(range(0, depth - 2, 2))
            # boundary last (relaxes the mask-DMA deadline), except the final
            # tile: there the boundary goes first so the kernel's exposed tail
            # chain ends on an unmasked pair
            pair_lo = [depth - 2] + body if T == TILE_ORDER[-1] and body else body + [depth - 2]
            for pi, lo in enumerate(pair_lo):
                boundary = lo == depth - 2
                eng = "a" if (boundary or pi % 2 == 0) else "v"
                work.append(
                    dict(T=T, lo=lo, boundary=boundary, eng=eng,
                         first=pi == 0, last=pi == len(pair_lo) - 1, depth=depth)
                )
        # re-balance: alternate interior pairs between DVE and ACT
        flip = True
        for w in work:
            if not w["boundary"]:
                w["eng"] = "v" if flip else "a"
                flip = not flip

        ops_tiles = {}
        n_mm2 = {}
        pts = {}

        def emit_mm1_exp(i):
            w = work[i]
            T, lo, boundary = w["T"], w["lo"], w["boundary"]
            sc = sc_ps.tile([BLK, 2 * QTW], f32, tag="sc")
            wid = (QTW, QTW // 2) if boundary else (QTW, QTW)
            for k, rg in ((0, 0), (1, D)):
                blk = lo + k
                if T == 0:
                    # tile 0 has short softmax rows: keep its scores in fp16
                    kk = kt0_s[rg : rg + D, blk * BLK : (blk + 1) * BLK]
                    qq = qt0_s[rg : rg + D, QTW - wid[k] : QTW]
                else:
                    kk = kt_s[rg : rg + D, blk * BLK : (blk + 1) * BLK]
                    qq = qt_s[rg : rg + D, T * QTW + (QTW - wid[k]) : (T + 1) * QTW]
                nc.tensor.matmul(
                    sc[:, k * QTW : k * QTW + wid[k]],
                    kk,
                    qq,
                    start=True,
                    stop=True,
                    tile_position=(rg, 0),
                )
            ew = QTW + wid[1]
            if w["eng"] == "v" and not boundary:
                pti = ppool.tile([BLK, 2 * QTW], i16, tag="pti")
                nc.vector.tensor_scalar(
                    pti[:, 0:ew], sc[:, 0:ew], SCH_A, SCH_B,
                    mybir.AluOpType.mult, mybir.AluOpType.add,
                )
                pt = pti[:].bitcast(f16)
            else:
                ptt = ppool.tile([BLK, 2 * QTW], f16, tag="pt")
                nc.scalar.activation(ptt[:, 0:ew], sc[:, 0:ew], Exp, scale=ACT_SCALE)
                pt = ptt[:]
            if boundary:
                # pt cols 0:768 (block lo full 512 + block hi tail 256) line up
                # exactly with the trimmed mask -- one 2x-mode multiply
                nc.vector.tensor_mul(pt[:, 0:MKW], pt[:, 0:MKW], mk_s[:])
            pts[i] = (pt, wid)

        def emit_mm2(i):
            w = work[i]
            T, lo, boundary = w["T"], w["lo"], w["boundary"]
            pt, wid = pts.pop(i)
            if T not in ops_tiles:
                ops_tiles[T] = out_ps.tile([65, QTW], f32, tag="ops", name=f"ops{T}")
                n_mm2[T] = 0
            ops = ops_tiles[T]
            # when the boundary (narrow) pair closes a tile, emit the full-
            # width matmul last so the stop flag covers every ops element
            for k in ((1, 0) if boundary and w["last"] else (0, 1)):
                blk = lo + k
                n_mm2[T] += 1
                nc.tensor.matmul(
                    ops[:, QTW - wid[k] : QTW],
                    vp_s[:, blk * 65 : (blk + 1) * 65],
                    pt[:, k * QTW : k * QTW + wid[k]],
                    start=(n_mm2[T] == 1),
                    stop=(n_mm2[T] == w["depth"]),
                )
            if w["last"]:
                osb = opool.tile([65, QTW], f16, tag="osb", name=f"osb{T}")
                if T == TILE_ORDER[-1]:
                    nc.scalar.copy(osb[:], ops[:])
                    nc.scalar.dma_start(out_d[:, T * QTW : (T + 1) * QTW], osb[:])
                else:
                    nc.vector.tensor_copy(osb[:], ops[:])
                    nc.sync.dma_start(out_d[:, T * QTW : (T + 1) * QTW], osb[:])
                del ops_tiles[T]

        # Emit in batches of two slots: the second pair's kt LDWEIGHTS can
        # pull ahead during the first pair's stream (its target rows are
        # free), hiding one of the two exposed weight loads per batch.
        for g in range(0, len(work) + LOOKAHEAD, 2):
            for j in (g, g + 1):
                if j < len(work):
                    emit_mm1_exp(j)
            for j in (g - LOOKAHEAD, g - LOOKAHEAD + 1):
                if 0 <= j < len(work):
                    emit_mm2(j)

    nc.compile()
    _prog_cache["nc"] = nc
    return nc


def _make_masks(h):
    """[128, 1024] fp16 multiplicative (1=keep, 0=masked) masks: two stacked
    tiles for the 2nd-to-last / last parity-kv loop positions of every q tile
    (relative diagonal offsets r = h and r = h + 2)."""
    tri = (np.arange(QTW)[None, :BLK] >= np.arange(BLK)[:, None]).astype(np.float16)
    full = np.zeros((BLK, BLK), dtype=np.float16)
    keep = np.ones((BLK, BLK), dtype=np.float16)

    def mask_for_r(r):
        cols = []
        for cb in range(QTW // BLK):
            if cb < r:
                cols.append(full)
            elif cb == r:
                cols.append(tri)
            else:
                cols.append(keep)
        return np.concatenate(cols, axis=1)  # [128, 512]

    # [128, 768]: lo-block mask (512) + last 256 cols of the hi-block mask
    # (its first 256 cols are never computed)
    return np.concatenate([mask_for_r(h), mask_for_r(h + 2)[:, 256:]], axis=1)


def kernel(query, key, value, padding):
    query = np.asarray(query, dtype=np.float32)
    key = np.asarray(key, dtype=np.float32)
    value = np.asarray(value, dtype=np.float32)
    padding = np.asarray(padding, dtype=bool)

    nc = _build_program()

    import ml_dtypes  # noqa: F401  (f8 dtype via mybir.dt.np)
    F8 = mybir.dt.np(mybir.dt.float8e4)

    in_maps = []
    for c in range(NCORES):
        b, h = divmod(c, 2)
        qt1 = np.ascontiguousarray(query[b].T)  # [64, 4096]
        qtd = np.concatenate([qt1, qt1], axis=0)  # [128, 4096] (row-tiling dup)
        qt = qtd.astype(F8)
        qt0 = qtd[:, 0:QTW].astype(np.float16)
        kT = key[b].T * KSCALE  # [64, 4096], pre-scaled
        blocks = [2 * i + h for i in range(PAR)]
        kt1 = np.ascontiguousarray(
            np.concatenate([kT[:, BLK * j : BLK * (j + 1)] for j in blocks], axis=1)
        )  # [64, 2048]
        ktd = np.concatenate([kt1, kt1], axis=0)  # [128, 2048] (row-tiling dup)
        kt = ktd.astype(F8)
        kt0 = ktd[:, 0 : 2 * BLK].astype(np.float16)
        vp = np.zeros((BLK, PAR * 65), dtype=np.float16)
        for i, j in enumerate(blocks):
            vblk = value[b, BLK * j : BLK * (j + 1), :].copy()
            pblk = padding[b, BLK * j : BLK * (j + 1)]
            vblk[pblk] = 0.0
            vp[:, 65 * i : 65 * i + 64] = vblk * (1.0 / 64.0)
            vp[:, 65 * i + 64] = np.where(pblk, 0.0, 1.0 / 64.0)
        in_maps.append(
            {"qt": qt, "kt": kt, "qt0": qt0, "kt0": kt0, "vp": vp, "mk": _make_masks(h)}
        )

    global _last_in_maps
    _last_in_maps = in_maps
    res = run_bass_kernel_spmd(nc, in_maps, list(range(NCORES)))

    out = np.empty((B, S, D), dtype=np.float32)
    for b in range(B):
        r0 = res.results[2 * b]["out"].astype(np.float64)
        r1 = res.results[2 * b + 1]["out"].astype(np.float64)
        num = r0[:64] + r1[:64]  # [64, 4096]
        den = r0[64] + r1[64]  # [4096]
        out[b] = (num / den).T.astype(np.float32)
    return out


# revision 39
# speedup vs baseline: 1.1858x; 1.1858x over previous
"""Causal attention (B=4, S=4096, D=64, fp32) on 8 Trainium2 NeuronCores.

Strategy
--------
Sharding: 2 cores per batch element; the two cores of a batch split the KV
blocks by parity (even / odd 128-row blocks). Each core computes, for every
query position of its batch, the *unnormalized* attention numerator and the
softmax denominator contribution of its own KV half. The host sums the two
halves and divides (exactly linear, since the softmax uses no max-subtraction:
scores/8 are bounded by ~|6| for N(0,1) inputs, so exp never overflows fp32).

Per-core device kernel (identical SPMD program; per-core behavior comes only
from input data):
  - scores^T layout: S_T[kv, q] = K @ Q^T in fp8 e4m3 (fp16 for the short-row
    q-tile 0, whose softmax rows are too short to average out fp8 score
    noise). Q^T/K^T are duplicated onto partitions 64-127 so each matmul pair
    runs concurrently in the 128x128 array via row tiling (tile_position).
    K^T is pre-scaled by log2(e) on the host so both exp paths below get a
    free affine. fp8 halves the dominant input bytes (the serial per-ring DMA
    bandwidth, ~60 GB/s, gates the pipeline start).
  - softmax exp is SPLIT across two engines (single-engine exp was the
    baseline bottleneck at ~42us busy):
      * ACT (scalar): exact exp via activation (scale folds the /8 and the
        log2e pre-scale), fp32 PSUM in -> fp16 SBUF out. Handles boundary
        (diagonal) pairs + every other interior pair. The exp table is
        preloaded during the DMA window by a dummy activation.
      * DVE (vector): Schraudolph exp for the other interior pairs:
        int16 = sc * 128 + 1024*(15+c)  (one tensor_scalar mult+add, fp32
        PSUM in), bitcast to fp16 = 2^y with linearly-interpolated mantissa
        (max ~3% relative error; averages out over the >=512-key rows that
        interior pairs serve; end-to-end rel err ~1.1e-2 vs 2e-2 budget,
        fp8 scores included).
  - the PE stream is SOFTWARE-PIPELINED with lookahead 4 and emitted in
    BATCHES OF TWO slots (two mm1 pairs, then two mm2 pairs): the PE queue
    is in-order, so emitting mm2(i) right after exp(i) would stall the
    queue and serialize the two exp engines; batching additionally lets the
    second mm1 pair's kt LDWEIGHTS pull ahead during the first pair's
    stream (its target rows are free) and lets mm2s run back-to-back
    without per-pair weight-load exposure (~7% measured win).
  - causal masking: the two KV blocks straddling the diagonal get one
    multiplicative 0/1 fp16 mask (2x DVE mode) applied to P after the exp;
    tiles are processed deepest-first with the boundary pair last, which
    relaxes the mask-DMA deadline to mid-kernel.
  - numerator+denominator: matmul(lhsT=[V/64 | 1/64] block [128,65], rhs=P
    [128,512]) accumulated over KV blocks in PSUM; row 64 is sum(P)/64 = the
    scaled softmax denominator. The 1/64 scale keeps the fp16 output DMA
    (half the bytes of fp32) from overflowing. Padded keys are zeroed in V
    AND the ones column.
  - input DMAs are spread over six queues (sync + scalar HWDGE, 4 SWDGE
    rings -- SWDGE siblings selected by renaming the queue on the emitted
    instruction) and ordered by first use.
Host: transposes Q/K, packs per-core inputs, combines/normalizes outputs.
"""

import math
import numpy as np
from contextlib import ExitStack

import concourse.tile as tile
from concourse import bacc, mybir
from concourse.bass_utils import run_bass_kernel_spmd

B, S, D = 4, 4096, 64
NCORES = 8
BLK = 128            # kv block rows
QTW = 512            # q tile width
NQT = S // QTW       # 8 q tiles
PAR = S // BLK // 2  # 16 kv blocks per parity half
WARMUP_MMS = 3       # dummy matmuls to open the PE HAM clock gate at startup
LOOKAHEAD = 3        # software pipeline depth (mm1 slots ahead of mm2)
BATCH = 3            # slots per emission wave (bounded by the 3 sc PSUM bufs)

KSCALE = math.log2(math.e)          # host pre-scale on K^T
ACT_SCALE = 0.125 / KSCALE          # activation: exp(sc * ACT_SCALE) = exp(s/8)
SCH_C = -0.0434
SCH_A = 128.0                       # 1024*log2e/8 / KSCALE
SCH_B = 1024.0 * (15.0 + SCH_C)

TILE_ORDER = [7, 6, 5, 4, 3, 2, 0, 1]
MKW = QTW + QTW // 2  # trimmed mask width (the middle 256 cols were never read)

_prog_cache = {}


def _build_program():
    if "nc" in _prog_cache:
        return _prog_cache["nc"]
    nc = bacc.Bacc(
        "TRN2", target_bir_lowering=False, debug=False, num_devices=NCORES,
        num_swdge_queues=4,
    )
    f32, f16, i16 = mybir.dt.float32, mybir.dt.float16, mybir.dt.int16
    Exp = mybir.ActivationFunctionType.Exp

    f8 = mybir.dt.float8e4
    qt_d = nc.dram_tensor("qt", [2 * D, S], f8, kind="ExternalInput").ap()
    kt_d = nc.dram_tensor("kt", [2 * D, PAR * BLK], f8, kind="ExternalInput").ap()
    qt0_d = nc.dram_tensor("qt0", [2 * D, QTW], f16, kind="ExternalInput").ap()
    kt0_d = nc.dram_tensor("kt0", [2 * D, 2 * BLK], f16, kind="ExternalInput").ap()
    vp_d = nc.dram_tensor("vp", [BLK, PAR * 65], f16, kind="ExternalInput").ap()
    mk_d = nc.dram_tensor("mk", [BLK, MKW], f16, kind="ExternalInput").ap()
    out_d = nc.dram_tensor("out", [65, S], f16, kind="ExternalOutput").ap()

    def gdma(q, out, in_):
        # SWDGE multi-queue: dma_start pins queue 0 ("qPoolDynamic"); route
        # to a sibling ring by renaming the queue on the emitted instruction
        # (queues 1-3 are declared via num_swdge_queues=4).
        inst = nc.gpsimd.dma_start(out, in_)
        if q:
            inst.ins.queue = f"qPoolDynamic{q}"
        return inst

    with tile.TileContext(nc) as tc, ExitStack() as ctx:
        const = ctx.enter_context(tc.tile_pool(name="const", bufs=1))
        ppool = ctx.enter_context(tc.tile_pool(name="pp", bufs=8))
        opool = ctx.enter_context(tc.tile_pool(name="op", bufs=3))
        sc_ps = ctx.enter_context(tc.tile_pool(name="scps", bufs=3, space="PSUM"))
        out_ps = ctx.enter_context(tc.tile_pool(name="ops", bufs=2, space="PSUM"))

        # Input DMAs spread over six queues (sync HWDGE, scalar HWDGE, and
        # four SWDGE rings), ordered by first use: tiles run deepest-first,
        # so kt/vp prefixes and qt[t7] gate the pipeline start; the mask is
        # only needed at each tile's final (boundary) pair.
        mk_s = const.tile([BLK, MKW], f16)
        kt_s = const.tile([2 * D, PAR * BLK], f8)
        vp_s = const.tile([BLK, PAR * 65], f16)
        qt_s = const.tile([2 * D, S], f8)
        qt0_s = const.tile([2 * D, QTW], f16)
        kt0_s = const.tile([2 * D, 2 * BLK], f16)

        def qtchunk(t):
            return qt_s[:, t * QTW : (t + 1) * QTW], qt_d[:, t * QTW : (t + 1) * QTW]

        nc.scalar.dma_start(kt_s[:, 0:512], kt_d[:, 0:512])
        nc.sync.dma_start(*qtchunk(7))
        gdma(1, vp_s[:, 0 : 8 * 65], vp_d[:, 0 : 8 * 65])
        gdma(2, kt_s[:, 1280:2048], kt_d[:, 1280:2048])
        gdma(3, vp_s[:, 8 * 65 :], vp_d[:, 8 * 65 :])
        nc.scalar.dma_start(kt_s[:, 512:1280], kt_d[:, 512:1280])
        nc.sync.dma_start(*qtchunk(6))
        gdma(2, *qtchunk(5))
        gdma(1, *qtchunk(4))
        nc.scalar.dma_start(mk_s[:], mk_d[:])
        nc.sync.dma_start(*qtchunk(3))
        gdma(3, *qtchunk(2))
        gdma(2, *qtchunk(1))
        nc.sync.dma_start(*qtchunk(0))
        gdma(1, qt0_s[:], qt0_d[:])
        gdma(3, kt0_s[:], kt0_d[:])

        # PE warmup (HAM clock gate needs ~3.4us busy) + ACT exp-table
        # preload, both during the input-DMA window.
        wsrc = const.tile([BLK, QTW], f16, name="wsrc")
        nc.vector.memset(wsrc[:], 0.0)
        wact = const.tile([BLK, 8], f16, name="wact")
        nc.scalar.activation(wact[:], wsrc[:, 0:8], Exp, scale=ACT_SCALE)
        wps = sc_ps.tile([BLK, 2 * QTW], f32, tag="sc", name="wps")
        for _ in range(WARMUP_MMS):
            nc.tensor.matmul(wps[:, 0:QTW], wsrc[:, 0:BLK], wsrc[:], start=True, stop=True)

        # Flat pair work list across tiles. Each entry:
        # (T, lo, boundary, engine, first_of_tile, last_of_tile, depth)
        work = []
        for T in TILE_ORDER:
            depth = 2 * T + 2
            body = list(range(0, depth - 2, 2))
            # boundary last (relaxes the mask-DMA deadline), except the final
            # tile: there the boundary goes first so the kernel's exposed tail
            # chain ends on an unmasked pair
            pair_lo = [depth - 2] + body if T == TILE_ORDER[-1] and body else body + [depth - 2]
            for pi, lo in enumerate(pair_lo):
                boundary = lo == depth - 2
                eng = "a" if (boundary or pi % 2 == 0) else "v"
                work.append(
                    dict(T=T, lo=lo, boundary=boundary, eng=eng,
                         first=pi == 0, last=pi == len(pair_lo) - 1, depth=depth)
                )
        # re-balance: alternate interior pairs between DVE and ACT
        flip = True
        for w in work:
            if not w["boundary"]:
                w["eng"] = "v" if flip else "a"
                flip = not flip

        ops_tiles = {}
        n_mm2 = {}
        pts = {}

        def emit_mm1_exp(i):
            w = work[i]
            T, lo, boundary = w["T"], w["lo"], w["boundary"]
            sc = sc_ps.tile([BLK, 2 * QTW], f32, tag="sc")
            wid = (QTW, QTW // 2) if boundary else (QTW, QTW)
            for k, rg in ((0, 0), (1, D)):
                blk = lo + k
                if T == 0:
                    # tile 0 has short softmax rows: keep its scores in fp16
                    kk = kt0_s[rg : rg + D, blk * BLK : (blk + 1) * BLK]
                    qq = qt0_s[rg : rg + D, QTW - wid[k] : QTW]
                else:
                    kk = kt_s[rg : rg + D, blk * BLK : (blk + 1) * BLK]
                    qq = qt_s[rg : rg + D, T * QTW + (QTW - wid[k]) : (T + 1) * QTW]
                nc.tensor.matmul(
                    sc[:, k * QTW : k * QTW + wid[k]],
                    kk,
                    qq,
                    start=True,
                    stop=True,
                    tile_position=(rg, 0),
                )
            ew = QTW + wid[1]
            if w["eng"] == "v" and not boundary:
                pti = ppool.tile([BLK, 2 * QTW], i16, tag="pti")
                nc.vector.tensor_scalar(
                    pti[:, 0:ew], sc[:, 0:ew], SCH_A, SCH_B,
                    mybir.AluOpType.mult, mybir.AluOpType.add,
                )
                pt = pti[:].bitcast(f16)
            else:
                ptt = ppool.tile([BLK, 2 * QTW], f16, tag="pt")
                nc.scalar.activation(ptt[:, 0:ew], sc[:, 0:ew], Exp, scale=ACT_SCALE)
                pt = ptt[:]
            if boundary:
                # pt cols 0:768 (block lo full 512 + block hi tail 256) line up
                # exactly with the trimmed mask -- one 2x-mode multiply
                nc.vector.tensor_mul(pt[:, 0:MKW], pt[:, 0:MKW], mk_s[:])
            pts[i] = (pt, wid)

        def emit_mm2(i):
            w = work[i]
            T, lo, boundary = w["T"], w["lo"], w["boundary"]
            pt, wid = pts.pop(i)
            if T not in ops_tiles:
                ops_tiles[T] = out_ps.tile([65, QTW], f32, tag="ops", name=f"ops{T}")
                n_mm2[T] = 0
            ops = ops_tiles[T]
            # when the boundary (narrow) pair closes a tile, emit the full-
            # width matmul last so the stop flag covers every ops element
            for k in ((1, 0) if boundary and w["last"] else (0, 1)):
                blk = lo + k
                n_mm2[T] += 1
                nc.tensor.matmul(
                    ops[:, QTW - wid[k] : QTW],
                    vp_s[:, blk * 65 : (blk + 1) * 65],
                    pt[:, k * QTW : k * QTW + wid[k]],
                    start=(n_mm2[T] == 1),
                    stop=(n_mm2[T] == w["depth"]),
                )
            if w["last"]:
                osb = opool.tile([65, QTW], f16, tag="osb", name=f"osb{T}")
                if T == TILE_ORDER[-1]:
                    nc.scalar.copy(osb[:], ops[:])
                    nc.scalar.dma_start(out_d[:, T * QTW : (T + 1) * QTW], osb[:])
                else:
                    nc.vector.tensor_copy(osb[:], ops[:])
                    nc.sync.dma_start(out_d[:, T * QTW : (T + 1) * QTW], osb[:])
                del ops_tiles[T]

        # Emit in waves of BATCH slots: each subsequent mm1 pair's kt
        # LDWEIGHTS pulls ahead during the previous pair's stream (its
        # target rows are free), and the wave's mm2s then run back-to-back
        # without per-pair weight-load exposure.
        for g in range(0, len(work) + LOOKAHEAD, BATCH):
            for j in range(g, g + BATCH):
                if j < len(work):
                    emit_mm1_exp(j)
            for j in range(g - LOOKAHEAD, g - LOOKAHEAD + BATCH):
                if 0 <= j < len(work):
                    emit_mm2(j)

    nc.compile()
    _prog_cache["nc"] = nc
    return nc


def _make_masks(h):
    """[128, 1024] fp16 multiplicative (1=keep, 0=masked) masks: two stacked
    tiles for the 2nd-to-last / last parity-kv loop positions of every q tile
    (relative diagonal offsets r = h and r = h + 2)."""
    tri = (np.arange(QTW)[None, :BLK] >= np.arange(BLK)[:, None]).astype(np.float16)
    full = np.zeros((BLK, BLK), dtype=np.float16)
    keep = np.ones((BLK, BLK), dtype=np.float16)

    def mask_for_r(r):
        cols = []
        for cb in range(QTW // BLK):
            if cb < r:
                cols.append(full)
            elif cb == r:
                cols.append(tri)
            else:
                cols.append(keep)
        return np.concatenate(cols, axis=1)  # [128, 512]

    # [128, 768]: lo-block mask (512) + last 256 cols of the hi-block mask
    # (its first 256 cols are never computed)
    return np.concatenate([mask_for_r(h), mask_for_r(h + 2)[:, 256:]], axis=1)


def kernel(query, key, value, padding):
    query = np.asarray(query, dtype=np.float32)
    key = np.asarray(key, dtype=np.float32)
    value = np.asarray(value, dtype=np.float32)
    padding = np.asarray(padding, dtype=bool)

    nc = _build_program()

    import ml_dtypes  # noqa: F401  (f8 dtype via mybir.dt.np)
    F8 = mybir.dt.np(mybir.dt.float8e4)

    in_maps = []
    for c in range(NCORES):
        b, h = divmod(c, 2)
        qt1 = np.ascontiguousarray(query[b].T)  # [64, 4096]
        qtd = np.concatenate([qt1, qt1], axis=0)  # [128, 4096] (row-tiling dup)
        qt = qtd.astype(F8)
        qt0 = qtd[:, 0:QTW].astype(np.float16)
        kT = key[b].T * KSCALE  # [64, 4096], pre-scaled
        blocks = [2 * i + h for i in range(PAR)]
        kt1 = np.ascontiguousarray(
            np.concatenate([kT[:, BLK * j : BLK * (j + 1)] for j in blocks], axis=1)
        )  # [64, 2048]
        ktd = np.concatenate([kt1, kt1], axis=0)  # [128, 2048] (row-tiling dup)
        kt = ktd.astype(F8)
        kt0 = ktd[:, 0 : 2 * BLK].astype(np.float16)
        vp = np.zeros((BLK, PAR * 65), dtype=np.float16)
        for i, j in enumerate(blocks):
            vblk = value[b, BLK * j : BLK * (j + 1), :].copy()
            pblk = padding[b, BLK * j : BLK * (j + 1)]
            vblk[pblk] = 0.0
            vp[:, 65 * i : 65 * i + 64] = vblk * (1.0 / 64.0)
            vp[:, 65 * i + 64] = np.where(pblk, 0.0, 1.0 / 64.0)
        in_maps.append(
            {"qt": qt, "kt": kt, "qt0": qt0, "kt0": kt0, "vp": vp, "mk": _make_masks(h)}
        )

    global _last_in_maps
    _last_in_maps = in_maps
    res = run_bass_kernel_spmd(nc, in_maps, list(range(NCORES)))

    out = np.empty((B, S, D), dtype=np.float32)
    for b in range(B):
        r0 = res.results[2 * b]["out"].astype(np.float64)
        r1 = res.results[2 * b + 1]["out"].astype(np.float64)
        num = r0[:64] + r1[:64]  # [64, 4096]
        den = r0[64] + r1[64]  # [4096]
        out[b] = (num / den).T.astype(np.float32)
    return out
